# revision 9
# baseline (speedup 1.0000x reference)
"""Multi-head attention (B=2, N=2048, D=1024, H=16) on 8 Trainium2 cores.

Sharding: data-parallel over batch (2) x tensor-parallel over head groups (4).
Core c handles batch c//4, heads 4*(c%4) .. 4*(c%4)+3 (ch=256 channels).

Per-core kernel (all matmuls bf16; psum f32):
  front:   kT/qT column chains  [ch, tok] = (W^T x^T) + bias (DVE add)
           v chains             [tok, ch] = x Wv + bv (ones-row matmul)
  pipeline over global slot s = 16*qt + kt2 (key ptile of 128):
    S^T[k, q]   4 matmuls/slot  lhsT=kT block [64,128], rhs=qT [64,512] (ap 512)
    exp         2 ACT ops/slot  (ap 1024: 2 heads x 512 queries) -> pt bf16
    P.V (s-2)   lhsT=pt block [128k,128q] (stationary!), rhs=v [128,64]
                -> O accum [128q, 64ch] (ap 64); sums via rhs=ones [128,1] (ap 1)
  per qt: recip(sums) -> normalize-drain (DVE tensor_scalar mul psum->bf16)
          -> DMA-xbar transpose O -> oT -> out-proj chains -> stage -> store
Host: out[b] = sum of the 4 group partials + b_o.
"""

import sys

sys.path.insert(0, "/opt/trn_rl_repo")

import numpy as np

B, N, D, H = 2, 2048, 1024, 16
SUB = D // H  # 64
GROUPS = 4
NH = H // GROUPS  # 4 local heads
CH = NH * SUB  # 256 local channels
NCORES = 8
KT = D // 128  # 8 contraction ptiles
TOKT = N // 128  # 16 token/key ptiles
QT = N // 512  # 4 query tiles
SCALE = SUB ** -0.5


def build_nc(name="mha2"):
    import concourse.mybir as mybir
    from concourse import bacc
    from concourse.tile import TileContext

    f32 = mybir.dt.float32
    bf16 = mybir.dt.bfloat16
    Exp = mybir.ActivationFunctionType.Exp

    DK, NT, DO, ch, nh = D, N, D, CH, NH
    NSLOT = QT * TOKT  # 64

    nc = bacc.Bacc(None, name=name)
    xT = nc.dram_tensor("xT", [DK, NT], bf16, kind="ExternalInput")
    wq = nc.dram_tensor("wq", [DK, ch], bf16, kind="ExternalInput")
    wk = nc.dram_tensor("wk", [DK, ch], bf16, kind="ExternalInput")
    wv = nc.dram_tensor("wv", [DK, ch], bf16, kind="ExternalInput")
    wvb = nc.dram_tensor("wvb", [1, ch], bf16, kind="ExternalInput")
    wo = nc.dram_tensor("wo", [ch, DO], bf16, kind="ExternalInput")
    bqk = nc.dram_tensor("bqk", [128, 2, 2], f32, kind="ExternalInput")
    ones_row = nc.dram_tensor("ones_row", [1, 128], bf16, kind="ExternalInput")
    ones_col = nc.dram_tensor("ones_col", [128, 1], bf16, kind="ExternalInput")
    ident = nc.dram_tensor("ident", [128, 128], bf16, kind="ExternalInput")
    out = nc.dram_tensor("out", [NT, DO], bf16, kind="ExternalOutput")

    with TileContext(nc) as tc:
        with tc.tile_pool(name="persist", bufs=1) as pp:
            xt = pp.tile([128, KT, NT], bf16)
            wq_sb = pp.tile([128, KT, ch], bf16)
            wk_sb = pp.tile([128, KT, ch], bf16)
            wv_sb = pp.tile([128, KT, ch], bf16)
            wvb_sb = pp.tile([1, ch], bf16)
            wo_sb = pp.tile([128, 2, DO], bf16)
            bqk_sb = pp.tile([128, 2, 2], f32)
            onr = pp.tile([1, 128], bf16)
            onc = pp.tile([128, 1], bf16)
            ident_sb = pp.tile([128, 128], bf16)
            qT_sb = pp.tile([128, 2, NT], bf16)
            kT_sb = pp.tile([128, 2, NT], bf16)
            v_sb = pp.tile([128, TOKT, ch], bf16)
            wu_src = pp.tile([128, 512], bf16)

            # warmup source (no DMA needed -> PE can ramp while x loads)
            nc.vector.memset(wu_src[:], 0.0)

            # input DMAs: x panel 0 and wk/wq arrive as interleaved
            # sub-chunks so the front chains can run in kt-lockstep
            def x_sub(kp):
                nc.sync.dma_start(
                    xt[:, 2 * kp : 2 * kp + 2, 0:512],
                    xT[256 * kp : 256 * (kp + 1), 0:512].rearrange(
                        "(kt p) c -> p kt c", p=128
                    ),
                )

            def w_half(dst, src, h):
                nc.sync.dma_start(
                    dst[:, 4 * h : 4 * h + 4, :],
                    src[512 * h : 512 * (h + 1), :].rearrange(
                        "(kt p) c -> p kt c", p=128
                    ),
                )

            x_sub(0)
            w_half(wk_sb, wk, 0)
            x_sub(1)
            w_half(wq_sb, wq, 0)
            w_half(wk_sb, wk, 1)
            x_sub(2)
            w_half(wq_sb, wq, 1)
            x_sub(3)
            nc.sync.dma_start(bqk_sb[:], bqk[:])
            nc.sync.dma_start(
                wv_sb[:], wv.rearrange("(kt p) c -> p kt c", p=128)
            )
            nc.sync.dma_start(wvb_sb[:], wvb[:])
            nc.sync.dma_start(onr[:], ones_row[:])
            nc.sync.dma_start(onc[:], ones_col[:])
            nc.sync.dma_start(ident_sb[:], ident[:])
            nc.sync.dma_start(
                xt[:, :, 512:1024],
                xT[:, 512:1024].rearrange("(kt p) c -> p kt c", p=128),
            )
            nc.sync.dma_start(
                wo_sb[:], wo.rearrange("(cb p) d -> p cb d", p=128)
            )
            nc.sync.dma_start(
                xt[:, :, 1024:1536],
                xT[:, 1024:1536].rearrange("(kt p) c -> p kt c", p=128),
            )
            nc.sync.dma_start(
                xt[:, :, 1536:2048],
                xT[:, 1536:2048].rearrange("(kt p) c -> p kt c", p=128),
            )

            # ---------------- chain emitters (front + streamed) ----------------
            def kq_chain(dst, w_sb, which, mt, col, pool, tag):
                """dst[:, mt, col*512:+512] = (W^T @ x^T) + bias."""
                ps = pool.tile([128, 512], f32, name=f"c{tag}", tag=tag)
                for kt in range(KT):
                    nc.tensor.matmul(
                        ps[:],
                        lhsT=w_sb[:, kt, mt * 128 : (mt + 1) * 128],
                        rhs=xt[:, kt, col * 512 : (col + 1) * 512],
                        start=(kt == 0),
                        stop=(kt == KT - 1),
                    )
                nc.vector.tensor_scalar_add(
                    dst[:, mt, col * 512 : (col + 1) * 512],
                    ps[:],
                    bqk_sb[:, mt, which : which + 1],
                )

            def v_chain(t, pool, tag):
                """v_sb[:, t, :] = (x @ Wv) + bv."""
                ps = pool.tile([128, ch], f32, name=f"v{tag}", tag=tag)
                for kt in range(KT):
                    nc.tensor.matmul(
                        ps[:],
                        lhsT=xt[:, kt, t * 128 : (t + 1) * 128],
                        rhs=wv_sb[:, kt, :],
                        start=(kt == 0),
                        stop=False,
                    )
                nc.tensor.matmul(
                    ps[:], lhsT=onr[:, 0:128], rhs=wvb_sb[:], start=False, stop=True
                )
                nc.vector.tensor_copy(v_sb[:, t, :], ps[:])

            # ---------------- front: warmup + minimal chains ----------------
            with tc.tile_pool(name="fpA", bufs=4, space="PSUM") as fpA, \
                 tc.tile_pool(name="fpW", bufs=1, space="PSUM") as fpW:
                # chains take banks 0-3 in order: {kT0,qT0} land in banks 0-1
                # (= S slot 0's banks) and drain first; warmup gets bank 4
                fch = [(kT_sb, wk_sb, 1, 0), (qT_sb, wq_sb, 0, 0),
                       (kT_sb, wk_sb, 1, 1), (qT_sb, wq_sb, 0, 1)]
                fps = [fpA.tile([128, 512], f32, name="fc", tag="fa")
                       for _ in fch]
                wu = fpW.tile([128, 512], f32, name="wu", tag="fw")
                for _ in range(4):
                    nc.tensor.matmul(
                        wu[:], lhsT=wu_src[:, 0:128], rhs=wu_src[:],
                        start=True, stop=True,
                    )

                def fmm(ci, kt):
                    dst, w_sb, which, mt = fch[ci]
                    nc.tensor.matmul(
                        fps[ci][:],
                        lhsT=w_sb[:, kt, mt * 128 : (mt + 1) * 128],
                        rhs=xt[:, kt, 0:512],
                        start=(kt == 0),
                        stop=(kt == KT - 1),
                    )

                # kt-lockstep while x sub-chunks arrive, then staggered finish
                # so the chains drain one at a time instead of all at once
                for kt in range(KT - 2):
                    for ci in range(4):
                        fmm(ci, kt)
                for ci in range(4):
                    fmm(ci, KT - 2)
                    fmm(ci, KT - 1)
                    dst, w_sb, which, mt = fch[ci]
                    nc.vector.tensor_scalar_add(
                        dst[:, mt, 0:512], fps[ci][:],
                        bqk_sb[:, mt, which : which + 1],
                    )

            # ---------------- main pipeline ----------------
            with tc.tile_pool(name="stp", bufs=2, space="PSUM") as stp, \
                 tc.tile_pool(name="obp", bufs=1, space="PSUM") as obp, \
                 tc.tile_pool(name="smp", bufs=1, space="PSUM") as smp, \
                 tc.tile_pool(name="acc", bufs=1, space="PSUM") as acc, \
                 tc.tile_pool(name="ptp", bufs=10) as ptp, \
                 tc.tile_pool(name="osb", bufs=2) as osb, \
                 tc.tile_pool(name="otp", bufs=16) as otp, \
                 tc.tile_pool(name="rcp", bufs=2) as rcp, \
                 tc.tile_pool(name="stg", bufs=8) as stg:

                pt_t = {}     # (s, half) -> pt tile
                oacc_t = {}   # qt -> [128, 16, 64] accum tile
                sm_t = {}     # qt -> [128, 16] sums tile
                O_t = {}      # qt -> [128, 4, 256] normalized O (bf16)
                recip_t = {}  # qt -> [128, 16] f32
                oT_t = {}     # (qt, qb, cb) -> [128, 128] bf16
                stage_t = {}  # tt -> [128, DO] f32

                def emit_S(s):
                    qt, kt2 = divmod(s, TOKT)
                    for half in range(2):
                        st = stp.tile([128, 2, 512], f32, name="st", tag="st")
                        for hh in range(2):
                            nc.tensor.matmul(
                                st[:, hh, :],
                                lhsT=kT_sb[
                                    64 * hh : 64 * hh + 64,
                                    half,
                                    kt2 * 128 : (kt2 + 1) * 128,
                                ],
                                rhs=qT_sb[
                                    64 * hh : 64 * hh + 64,
                                    half,
                                    qt * 512 : (qt + 1) * 512,
                                ],
                                start=True,
                                stop=True,
                            )
                        pt = ptp.tile([128, 2, 512], bf16, name="pt", tag="pt")
                        nc.scalar.activation(pt[:], st[:], Exp, scale=SCALE)
                        pt_t[(s, half)] = pt

                def emit_PV(s):
                    qt, kt2 = divmod(s, TOKT)
                    first, last = kt2 == 0, kt2 == TOKT - 1
                    if first:
                        oacc_t[qt] = obp.tile([128, 4 * nh, SUB], f32, name="ob", tag="ob")
                        sm_t[qt] = smp.tile([128, 4 * nh], f32, name="sm", tag="sm")
                        # start=True lazily zeroes the whole 2KB psum zero
                        # region, so open each bank exactly once with a zero
                        # matmul; all accumulating chains use start=False.
                        for half8 in range(2):
                            nc.tensor.matmul(
                                oacc_t[qt][:, 8 * half8 : 8 * half8 + 8, :],
                                lhsT=wu_src[:, 0:128],
                                rhs=wu_src[:, 0:512],
                                start=True,
                                stop=False,
                                skip_group_check=True,
                            )
                        nc.tensor.matmul(
                            sm_t[qt][:],
                            lhsT=wu_src[:, 0:128],
                            rhs=wu_src[:, 0:16],
                            start=True,
                            stop=False,
                            skip_group_check=True,
                        )
                    ob, sm = oacc_t[qt], sm_t[qt]
                    for qb in range(4):
                        for h in range(nh):
                            half, hh = divmod(h, 2)
                            ptb = pt_t[(s, half)][:, hh, qb * 128 : (qb + 1) * 128]
                            c = qb * nh + h
                            nc.tensor.matmul(
                                ob[:, c, :],
                                lhsT=ptb,
                                rhs=v_sb[:, kt2, h * SUB : (h + 1) * SUB],
                                start=False,
                                stop=last,
                                skip_group_check=True,
                            )
                            nc.tensor.matmul(
                                sm[:, c : c + 1],
                                lhsT=ptb,
                                rhs=onc[:],
                                start=False,
                                stop=last,
                                skip_group_check=True,
                            )
                    if last:
                        del pt_t[(s, 0)], pt_t[(s, 1)]

                mult = mybir.AluOpType.mult

                def emit_norm(qt):
                    """recip + batched normalize-drain + xbar transposes."""
                    rc = rcp.tile([128, 4 * nh, 1], f32, name="rc", tag="rc")
                    nc.vector.reciprocal(rc[:, :, 0], sm_t[qt][:])
                    recip_t[qt] = rc
                    O = osb.tile([128, 4 * nh, SUB], bf16, name="O", tag="O")
                    O_t[qt] = O
                    tail = qt == QT - 1
                    for qb in range(4):
                        nc.vector.tensor_tensor(
                            out=O[:, qb * nh : (qb + 1) * nh, :],
                            in0=oacc_t[qt][:, qb * nh : (qb + 1) * nh, :],
                            in1=rc[:, qb * nh : (qb + 1) * nh, :].broadcast_to(
                                [128, nh, SUB]
                            ),
                            op=mult,
                        )
                        if tail:
                            # PE is idle in the tail: transpose there (psum)
                            # instead of eating HWDGE issue serialization
                            trt = stp.tile([128, 2, 1024], bf16, name="st", tag="st")
                            for cb in range(2):
                                ot = otp.tile([128, 128], bf16, name="oT", tag="oT")
                                nc.tensor.transpose(
                                    trt[:, cb, 0:128],
                                    O[:, qb * nh + 2 * cb : qb * nh + 2 * cb + 2, :],
                                    ident_sb[:],
                                )
                                nc.vector.tensor_copy(ot[:], trt[:, cb, 0:128])
                                oT_t[(qt, qb, cb)] = ot
                        else:
                            for cb in range(2):
                                ot = otp.tile([128, 128], bf16, name="oT", tag="oT")
                                nc.sync.dma_start_transpose(
                                    ot[:], O[:, qb * nh + 2 * cb : qb * nh + 2 * cb + 2, :]
                                )
                                oT_t[(qt, qb, cb)] = ot

                def emit_op(qt, qb, nt, big=False, store_eng=None, tail=False):
                    """out-proj piece for tok-tile tt=4qt+qb, do-half nt."""
                    tt = 4 * qt + qb
                    if big:  # tail: reuse a freed S psum slot
                        ps = stp.tile([128, 2, 512], f32, name="st", tag="st")[:, 0, :]
                    else:
                        ps = acc.tile([128, 512], f32, name="op", tag="acc")
                    for cb in range(2):
                        nc.tensor.matmul(
                            ps[:],
                            lhsT=oT_t[(qt, qb, cb)][:],
                            rhs=wo_sb[:, cb, nt * 512 : (nt + 1) * 512],
                            start=(cb == 0),
                            stop=(cb == 1),
                        )
                    if tail:
                        # batch both do-halves into one store per tok-tile;
                        # alternate copies ACT/DVE (both have tail slack)
                        if nt == 0:
                            stage_t[tt] = stg.tile(
                                [128, 2, 512], bf16, name="s2", tag="s2"
                            )
                        if (2 * qb + nt) % 2:
                            nc.scalar.copy(stage_t[tt][:, nt, :], ps[:])
                        else:
                            nc.vector.tensor_copy(stage_t[tt][:, nt, :], ps[:])
                        if nt == 1:
                            (store_eng or nc.sync).dma_start(
                                out[tt * 128 : (tt + 1) * 128, :],
                                stage_t[tt][:].rearrange("p a b -> p (a b)"),
                            )
                        return
                    sg = stg.tile([128, 512], bf16, name="sg", tag="sg")
                    nc.vector.tensor_copy(sg[:], ps[:])
                    (store_eng or nc.sync).dma_start(
                        out[tt * 128 : (tt + 1) * 128, nt * 512 : (nt + 1) * 512],
                        sg[:],
                    )

                def emit_opq(qt, qb, nq):
                    """quarter out-proj piece: do range [nq*256, nq*256+256)."""
                    tt = 4 * qt + qb
                    ps = acc.tile([128, 256], f32, name="opq", tag="acc")
                    for cb in range(2):
                        nc.tensor.matmul(
                            ps[:],
                            lhsT=oT_t[(qt, qb, cb)][:],
                            rhs=wo_sb[:, cb, nq * 256 : (nq + 1) * 256],
                            start=(cb == 0),
                            stop=(cb == 1),
                        )
                    k, sub = divmod(nq, 2)
                    if sub == 0:
                        stage_t[(tt, k)] = stg.tile(
                            [128, 512], bf16, name="sg", tag="sg"
                        )
                    nc.vector.tensor_copy(
                        stage_t[(tt, k)][:, sub * 256 : (sub + 1) * 256], ps[:]
                    )
                    if sub == 1:
                        nc.sync.dma_start(
                            out[tt * 128 : (tt + 1) * 128, k * 512 : (k + 1) * 512],
                            stage_t[(tt, k)][:],
                        )

                # ------- stream schedule: slot -> list of thunks -------
                from collections import defaultdict

                stream = defaultdict(list)

                def sched(slot, fn):
                    stream[slot].append(fn)

                # v tiles: tile t needed by PV(t) at slot t+2
                for t in range(TOKT):
                    sched(t, lambda t=t: v_chain(t, acc, "acc"))
                # kT cols 1..3: col c needed by S(4c)
                for c2 in (1, 2, 3):
                    for mt in range(2):
                        sched(4 * (c2 - 1) + 2 * mt, lambda mt=mt, c2=c2:
                              kq_chain(kT_sb, wk_sb, 1, mt, c2, acc, "acc"))
                # qT for qt 1..3: needed by S(16qt)
                for q2 in (1, 2, 3):
                    for mt in range(2):
                        sched(16 * q2 - 6 + 2 * mt, lambda mt=mt, q2=q2:
                              kq_chain(qT_sb, wq_sb, 0, mt, q2, acc, "acc"))
                # out-proj quarters for qt spread across qt+1's slots so
                # every slot's PE load sits just under the ACT (exp) pace
                OPQ_SLOTS = (2, 3, 3, 4, 5, 6, 6, 7, 8, 9, 9, 10, 11, 14, 14, 15)
                for q2 in (0, 1, 2):
                    for i, (qb, nq) in enumerate(
                        [(qb, nq) for qb in range(4) for nq in range(4)]
                    ):
                        sched(16 * (q2 + 1) + OPQ_SLOTS[i],
                              lambda q2=q2, qb=qb, nq=nq: emit_opq(q2, qb, nq))

                # ------- the slot loop -------
                for s in range(NSLOT):
                    emit_S(s)
                    if s >= 2:
                        emit_PV(s - 2)
                    if s % 16 == 1 and s >= 17:
                        emit_norm(s // 16 - 1)
                    for fn in stream.pop(s, ()):
                        fn()

                # ------- tail -------
                emit_PV(NSLOT - 2)
                ptl = pt_t[(NSLOT - 1, 1)]
                emit_PV(NSLOT - 1)
                # keep the PE p-state hot through the norm/transpose latency;
                # lhsT=pt of the last slot anchors these after the last exp
                # (the scheduler sinks dependency-free instructions)
                wu2 = acc.tile([128, 512], f32, name="op", tag="acc")
                for _ in range(6):
                    nc.tensor.matmul(
                        wu2[:], lhsT=ptl[:, 0, 0:128], rhs=wu_src[:],
                        start=True, stop=True,
                    )
                emit_norm(QT - 1)
                for i, (qb, ntb) in enumerate(
                    [(qb, ntb) for qb in range(4) for ntb in range(2)]
                ):
                    emit_op(QT - 1, qb, ntb, big=(i % 3 != 0),
                            store_eng=(nc.scalar if qb % 2 else nc.sync), tail=True)
    nc.finalize()
    return nc


def make_in_maps(x, W_qkv, b_qkv, W_o):
    """Shard full inputs into per-core input maps (core c: batch c//4, group c%4)."""
    import ml_dtypes

    bf = ml_dtypes.bfloat16
    x = np.asarray(x, dtype=np.float32)
    W_qkv = np.asarray(W_qkv, dtype=np.float32)
    b_qkv = np.asarray(b_qkv, dtype=np.float32)
    W_o = np.asarray(W_o, dtype=np.float32)
    in_maps = []
    for c in range(NCORES):
        b, g = divmod(c, GROUPS)
        cols = slice(CH * g, CH * (g + 1))
        bq = b_qkv[0 * D : 1 * D][cols]
        bk = b_qkv[1 * D : 2 * D][cols]
        bv = b_qkv[2 * D : 3 * D][cols]
        bqk_t = np.empty((128, 2, 2), dtype=np.float32)
        for mt in range(2):
            bqk_t[:, mt, 0] = bq[mt * 128 : (mt + 1) * 128]
            bqk_t[:, mt, 1] = bk[mt * 128 : (mt + 1) * 128]
        m = {
            "xT": np.ascontiguousarray(x[b].T).astype(bf),
            "wq": np.ascontiguousarray(W_qkv[:, 0 * D : 1 * D][:, cols]).astype(bf),
            "wk": np.ascontiguousarray(W_qkv[:, 1 * D : 2 * D][:, cols]).astype(bf),
            "wv": np.ascontiguousarray(W_qkv[:, 2 * D : 3 * D][:, cols]).astype(bf),
            "wvb": np.ascontiguousarray(bv[None, :]).astype(bf),
            "wo": np.ascontiguousarray(W_o[cols, :]).astype(bf),
            "bqk": bqk_t,
            "ones_row": np.ones((1, 128), dtype=bf),
            "ones_col": np.ones((128, 1), dtype=bf),
            "ident": np.eye(128, dtype=np.float32).astype(bf),
        }
        in_maps.append(m)
    return in_maps


_NC = None


def get_nc():
    global _NC
    if _NC is None:
        _NC = build_nc()
    return _NC


def kernel(x, W_qkv, b_qkv, W_o, b_o):
    from concourse import bass_utils

    b_o = np.asarray(b_o, dtype=np.float32)
    in_maps = make_in_maps(x, W_qkv, b_qkv, W_o)
    res = bass_utils.run_bass_kernel_spmd(get_nc(), in_maps, core_ids=list(range(NCORES)))
    out = np.empty((B, N, D), dtype=np.float32)
    for b in range(B):
        acc = np.asarray(res.results[4 * b]["out"], dtype=np.float32)
        for g in range(1, GROUPS):
            acc = acc + np.asarray(res.results[4 * b + g]["out"], dtype=np.float32)
        out[b] = acc + b_o
    return out


# revision 10
# speedup vs baseline: 1.0091x; 1.0091x over previous
"""Multi-head attention (B=2, N=2048, D=1024, H=16) on 8 Trainium2 cores.

Sharding: data-parallel over batch (2) x tensor-parallel over head groups (4).
Core c handles batch c//4, heads 4*(c%4) .. 4*(c%4)+3 (ch=256 channels).

Per-core kernel (all matmuls bf16; psum f32):
  front:   kT/qT column chains  [ch, tok] = (W^T x^T) + bias (DVE add)
           v chains             [tok, ch] = x Wv + bv (ones-row matmul)
  pipeline over global slot s = 16*qt + kt2 (key ptile of 128):
    S^T[k, q]   4 matmuls/slot  lhsT=kT block [64,128], rhs=qT [64,512] (ap 512)
    exp         2 ACT ops/slot  (ap 1024: 2 heads x 512 queries) -> pt bf16
    P.V (s-2)   lhsT=pt block [128k,128q] (stationary!), rhs=v [128,64]
                -> O accum [128q, 64ch] (ap 64); sums via rhs=ones [128,1] (ap 1)
  per qt: recip(sums) -> normalize-drain (DVE tensor_scalar mul psum->bf16)
          -> DMA-xbar transpose O -> oT -> out-proj chains -> stage -> store
Host: out[b] = sum of the 4 group partials + b_o.
"""

import sys

sys.path.insert(0, "/opt/trn_rl_repo")

import numpy as np

B, N, D, H = 2, 2048, 1024, 16
SUB = D // H  # 64
GROUPS = 4
NH = H // GROUPS  # 4 local heads
CH = NH * SUB  # 256 local channels
NCORES = 8
KT = D // 128  # 8 contraction ptiles
TOKT = N // 128  # 16 token/key ptiles
QT = N // 512  # 4 query tiles
SCALE = SUB ** -0.5


def build_nc(name="mha2"):
    import concourse.mybir as mybir
    from concourse import bacc
    from concourse.tile import TileContext

    f32 = mybir.dt.float32
    bf16 = mybir.dt.bfloat16
    Exp = mybir.ActivationFunctionType.Exp

    DK, NT, DO, ch, nh = D, N, D, CH, NH
    NSLOT = QT * TOKT  # 64

    nc = bacc.Bacc(None, name=name)
    xT = nc.dram_tensor("xT", [DK, NT], bf16, kind="ExternalInput")
    wq = nc.dram_tensor("wq", [DK, ch], bf16, kind="ExternalInput")
    wk = nc.dram_tensor("wk", [DK, ch], bf16, kind="ExternalInput")
    wv = nc.dram_tensor("wv", [DK, ch], bf16, kind="ExternalInput")
    wvb = nc.dram_tensor("wvb", [1, ch], bf16, kind="ExternalInput")
    wo = nc.dram_tensor("wo", [ch, DO], bf16, kind="ExternalInput")
    bqk = nc.dram_tensor("bqk", [128, 2, 2], f32, kind="ExternalInput")
    ones_row = nc.dram_tensor("ones_row", [1, 128], bf16, kind="ExternalInput")
    ones_col = nc.dram_tensor("ones_col", [128, 1], bf16, kind="ExternalInput")
    ident = nc.dram_tensor("ident", [128, 128], bf16, kind="ExternalInput")
    out = nc.dram_tensor("out", [NT, DO], bf16, kind="ExternalOutput")

    with TileContext(nc) as tc:
        with tc.tile_pool(name="persist", bufs=1) as pp:
            xt = pp.tile([128, KT, NT], bf16)
            wq_sb = pp.tile([128, KT, ch], bf16)
            wk_sb = pp.tile([128, KT, ch], bf16)
            wv_sb = pp.tile([128, KT, ch], bf16)
            wvb_sb = pp.tile([1, ch], bf16)
            wo_sb = pp.tile([128, 2, DO], bf16)
            bqk_sb = pp.tile([128, 2, 2], f32)
            onr = pp.tile([1, 128], bf16)
            onc = pp.tile([128, 1], bf16)
            ident_sb = pp.tile([128, 128], bf16)
            qT_sb = pp.tile([128, 2, NT], bf16)
            kT_sb = pp.tile([128, 2, NT], bf16)
            v_sb = pp.tile([128, TOKT, ch], bf16)
            wu_src = pp.tile([128, 512], bf16)

            # warmup source (no DMA needed -> PE can ramp while x loads)
            nc.vector.memset(wu_src[:], 0.0)

            # input DMAs: x panel 0 and wk/wq arrive as interleaved
            # sub-chunks so the front chains can run in kt-lockstep
            def x_sub(kp):
                nc.sync.dma_start(
                    xt[:, 2 * kp : 2 * kp + 2, 0:512],
                    xT[256 * kp : 256 * (kp + 1), 0:512].rearrange(
                        "(kt p) c -> p kt c", p=128
                    ),
                )

            def w_half(dst, src, h):
                nc.sync.dma_start(
                    dst[:, 4 * h : 4 * h + 4, :],
                    src[512 * h : 512 * (h + 1), :].rearrange(
                        "(kt p) c -> p kt c", p=128
                    ),
                )

            x_sub(0)
            w_half(wk_sb, wk, 0)
            x_sub(1)
            w_half(wq_sb, wq, 0)
            x_sub(2)
            w_half(wk_sb, wk, 1)
            x_sub(3)
            w_half(wq_sb, wq, 1)
            nc.sync.dma_start(bqk_sb[:], bqk[:])
            nc.sync.dma_start(
                wv_sb[:], wv.rearrange("(kt p) c -> p kt c", p=128)
            )
            nc.sync.dma_start(wvb_sb[:], wvb[:])
            nc.sync.dma_start(onr[:], ones_row[:])
            nc.sync.dma_start(onc[:], ones_col[:])
            nc.sync.dma_start(ident_sb[:], ident[:])
            nc.sync.dma_start(
                xt[:, :, 512:1024],
                xT[:, 512:1024].rearrange("(kt p) c -> p kt c", p=128),
            )
            nc.sync.dma_start(
                wo_sb[:], wo.rearrange("(cb p) d -> p cb d", p=128)
            )
            nc.sync.dma_start(
                xt[:, :, 1024:1536],
                xT[:, 1024:1536].rearrange("(kt p) c -> p kt c", p=128),
            )
            nc.sync.dma_start(
                xt[:, :, 1536:2048],
                xT[:, 1536:2048].rearrange("(kt p) c -> p kt c", p=128),
            )

            # ---------------- chain emitters (front + streamed) ----------------
            def kq_chain(dst, w_sb, which, mt, col, pool, tag):
                """dst[:, mt, col*512:+512] = (W^T @ x^T) + bias."""
                ps = pool.tile([128, 512], f32, name=f"c{tag}", tag=tag)
                for kt in range(KT):
                    nc.tensor.matmul(
                        ps[:],
                        lhsT=w_sb[:, kt, mt * 128 : (mt + 1) * 128],
                        rhs=xt[:, kt, col * 512 : (col + 1) * 512],
                        start=(kt == 0),
                        stop=(kt == KT - 1),
                    )
                nc.vector.tensor_scalar_add(
                    dst[:, mt, col * 512 : (col + 1) * 512],
                    ps[:],
                    bqk_sb[:, mt, which : which + 1],
                )

            def v_chain(t, pool, tag):
                """v_sb[:, t, :] = (x @ Wv) + bv."""
                ps = pool.tile([128, ch], f32, name=f"v{tag}", tag=tag)
                for kt in range(KT):
                    nc.tensor.matmul(
                        ps[:],
                        lhsT=xt[:, kt, t * 128 : (t + 1) * 128],
                        rhs=wv_sb[:, kt, :],
                        start=(kt == 0),
                        stop=False,
                    )
                nc.tensor.matmul(
                    ps[:], lhsT=onr[:, 0:128], rhs=wvb_sb[:], start=False, stop=True
                )
                nc.vector.tensor_copy(v_sb[:, t, :], ps[:])

            # ---------------- front: warmup + minimal chains ----------------
            with tc.tile_pool(name="fpA", bufs=4, space="PSUM") as fpA, \
                 tc.tile_pool(name="fpW", bufs=1, space="PSUM") as fpW:
                # chains take banks 0-3 in order: {kT0,qT0} land in banks 0-1
                # (= S slot 0's banks) and drain first; warmup gets bank 4
                fch = [(kT_sb, wk_sb, 1, 0), (qT_sb, wq_sb, 0, 0),
                       (kT_sb, wk_sb, 1, 1), (qT_sb, wq_sb, 0, 1)]
                fps = [fpA.tile([128, 512], f32, name="fc", tag="fa")
                       for _ in fch]
                wu = fpW.tile([128, 512], f32, name="wu", tag="fw")
                for _ in range(4):
                    nc.tensor.matmul(
                        wu[:], lhsT=wu_src[:, 0:128], rhs=wu_src[:],
                        start=True, stop=True,
                    )

                def fmm(ci, kt):
                    dst, w_sb, which, mt = fch[ci]
                    nc.tensor.matmul(
                        fps[ci][:],
                        lhsT=w_sb[:, kt, mt * 128 : (mt + 1) * 128],
                        rhs=xt[:, kt, 0:512],
                        start=(kt == 0),
                        stop=(kt == KT - 1),
                    )

                # kt-lockstep while x sub-chunks arrive, then staggered finish
                # so the chains drain one at a time instead of all at once
                for kt in range(KT - 2):
                    for ci in range(4):
                        fmm(ci, kt)
                for ci in range(4):
                    fmm(ci, KT - 2)
                    fmm(ci, KT - 1)
                    dst, w_sb, which, mt = fch[ci]
                    nc.vector.tensor_scalar_add(
                        dst[:, mt, 0:512], fps[ci][:],
                        bqk_sb[:, mt, which : which + 1],
                    )

            # ---------------- main pipeline ----------------
            with tc.tile_pool(name="stp", bufs=2, space="PSUM") as stp, \
                 tc.tile_pool(name="obp", bufs=1, space="PSUM") as obp, \
                 tc.tile_pool(name="smp", bufs=1, space="PSUM") as smp, \
                 tc.tile_pool(name="acc", bufs=1, space="PSUM") as acc, \
                 tc.tile_pool(name="ptp", bufs=10) as ptp, \
                 tc.tile_pool(name="osb", bufs=2) as osb, \
                 tc.tile_pool(name="otp", bufs=16) as otp, \
                 tc.tile_pool(name="rcp", bufs=2) as rcp, \
                 tc.tile_pool(name="stg", bufs=8) as stg:

                pt_t = {}     # (s, half) -> pt tile
                oacc_t = {}   # qt -> [128, 16, 64] accum tile
                sm_t = {}     # qt -> [128, 16] sums tile
                O_t = {}      # qt -> [128, 4, 256] normalized O (bf16)
                recip_t = {}  # qt -> [128, 16] f32
                oT_t = {}     # (qt, qb, cb) -> [128, 128] bf16
                stage_t = {}  # tt -> [128, DO] f32

                def emit_S(s):
                    qt, kt2 = divmod(s, TOKT)
                    for half in range(2):
                        st = stp.tile([128, 2, 512], f32, name="st", tag="st")
                        for hh in range(2):
                            nc.tensor.matmul(
                                st[:, hh, :],
                                lhsT=kT_sb[
                                    64 * hh : 64 * hh + 64,
                                    half,
                                    kt2 * 128 : (kt2 + 1) * 128,
                                ],
                                rhs=qT_sb[
                                    64 * hh : 64 * hh + 64,
                                    half,
                                    qt * 512 : (qt + 1) * 512,
                                ],
                                start=True,
                                stop=True,
                            )
                        pt = ptp.tile([128, 2, 512], bf16, name="pt", tag="pt")
                        nc.scalar.activation(pt[:], st[:], Exp, scale=SCALE)
                        pt_t[(s, half)] = pt

                def emit_PV(s):
                    qt, kt2 = divmod(s, TOKT)
                    first, last = kt2 == 0, kt2 == TOKT - 1
                    if first:
                        oacc_t[qt] = obp.tile([128, 4 * nh, SUB], f32, name="ob", tag="ob")
                        sm_t[qt] = smp.tile([128, 4 * nh], f32, name="sm", tag="sm")
                        # start=True lazily zeroes the whole 2KB psum zero
                        # region, so open each bank exactly once with a zero
                        # matmul; all accumulating chains use start=False.
                        for half8 in range(2):
                            nc.tensor.matmul(
                                oacc_t[qt][:, 8 * half8 : 8 * half8 + 8, :],
                                lhsT=wu_src[:, 0:128],
                                rhs=wu_src[:, 0:512],
                                start=True,
                                stop=False,
                                skip_group_check=True,
                            )
                        nc.tensor.matmul(
                            sm_t[qt][:],
                            lhsT=wu_src[:, 0:128],
                            rhs=wu_src[:, 0:16],
                            start=True,
                            stop=False,
                            skip_group_check=True,
                        )
                    ob, sm = oacc_t[qt], sm_t[qt]
                    for qb in range(4):
                        for h in range(nh):
                            half, hh = divmod(h, 2)
                            ptb = pt_t[(s, half)][:, hh, qb * 128 : (qb + 1) * 128]
                            c = qb * nh + h
                            nc.tensor.matmul(
                                ob[:, c, :],
                                lhsT=ptb,
                                rhs=v_sb[:, kt2, h * SUB : (h + 1) * SUB],
                                start=False,
                                stop=last,
                                skip_group_check=True,
                            )
                            nc.tensor.matmul(
                                sm[:, c : c + 1],
                                lhsT=ptb,
                                rhs=onc[:],
                                start=False,
                                stop=last,
                                skip_group_check=True,
                            )
                    if last:
                        del pt_t[(s, 0)], pt_t[(s, 1)]

                mult = mybir.AluOpType.mult

                def emit_norm(qt):
                    """recip + batched normalize-drain + xbar transposes."""
                    rc = rcp.tile([128, 4 * nh, 1], f32, name="rc", tag="rc")
                    nc.vector.reciprocal(rc[:, :, 0], sm_t[qt][:])
                    recip_t[qt] = rc
                    O = osb.tile([128, 4 * nh, SUB], bf16, name="O", tag="O")
                    O_t[qt] = O
                    tail = qt == QT - 1
                    for qb in range(4):
                        nc.vector.tensor_tensor(
                            out=O[:, qb * nh : (qb + 1) * nh, :],
                            in0=oacc_t[qt][:, qb * nh : (qb + 1) * nh, :],
                            in1=rc[:, qb * nh : (qb + 1) * nh, :].broadcast_to(
                                [128, nh, SUB]
                            ),
                            op=mult,
                        )
                        if tail:
                            # PE is idle in the tail: transpose there (psum)
                            # instead of eating HWDGE issue serialization
                            trt = stp.tile([128, 2, 1024], bf16, name="st", tag="st")
                            for cb in range(2):
                                ot = otp.tile([128, 128], bf16, name="oT", tag="oT")
                                nc.tensor.transpose(
                                    trt[:, cb, 0:128],
                                    O[:, qb * nh + 2 * cb : qb * nh + 2 * cb + 2, :],
                                    ident_sb[:],
                                )
                                nc.vector.tensor_copy(ot[:], trt[:, cb, 0:128])
                                oT_t[(qt, qb, cb)] = ot
                        else:
                            for cb in range(2):
                                ot = otp.tile([128, 128], bf16, name="oT", tag="oT")
                                nc.sync.dma_start_transpose(
                                    ot[:], O[:, qb * nh + 2 * cb : qb * nh + 2 * cb + 2, :]
                                )
                                oT_t[(qt, qb, cb)] = ot

                def emit_op(qt, qb, nt, big=False, store_eng=None, tail=False):
                    """out-proj piece for tok-tile tt=4qt+qb, do-half nt."""
                    tt = 4 * qt + qb
                    if big:  # tail: reuse a freed S psum slot
                        ps = stp.tile([128, 2, 512], f32, name="st", tag="st")[:, 0, :]
                    else:
                        ps = acc.tile([128, 512], f32, name="op", tag="acc")
                    for cb in range(2):
                        nc.tensor.matmul(
                            ps[:],
                            lhsT=oT_t[(qt, qb, cb)][:],
                            rhs=wo_sb[:, cb, nt * 512 : (nt + 1) * 512],
                            start=(cb == 0),
                            stop=(cb == 1),
                        )
                    if tail:
                        # batch both do-halves into one store per tok-tile;
                        # alternate copies ACT/DVE (both have tail slack)
                        if nt == 0:
                            stage_t[tt] = stg.tile(
                                [128, 2, 512], bf16, name="s2", tag="s2"
                            )
                        if (2 * qb + nt) % 2:
                            nc.scalar.copy(stage_t[tt][:, nt, :], ps[:])
                        else:
                            nc.vector.tensor_copy(stage_t[tt][:, nt, :], ps[:])
                        if nt == 1:
                            (store_eng or nc.sync).dma_start(
                                out[tt * 128 : (tt + 1) * 128, :],
                                stage_t[tt][:].rearrange("p a b -> p (a b)"),
                            )
                        return
                    sg = stg.tile([128, 512], bf16, name="sg", tag="sg")
                    nc.vector.tensor_copy(sg[:], ps[:])
                    (store_eng or nc.sync).dma_start(
                        out[tt * 128 : (tt + 1) * 128, nt * 512 : (nt + 1) * 512],
                        sg[:],
                    )

                def emit_opq(qt, qb, nq):
                    """quarter out-proj piece: do range [nq*256, nq*256+256)."""
                    tt = 4 * qt + qb
                    ps = acc.tile([128, 256], f32, name="opq", tag="acc")
                    for cb in range(2):
                        nc.tensor.matmul(
                            ps[:],
                            lhsT=oT_t[(qt, qb, cb)][:],
                            rhs=wo_sb[:, cb, nq * 256 : (nq + 1) * 256],
                            start=(cb == 0),
                            stop=(cb == 1),
                        )
                    k, sub = divmod(nq, 2)
                    if sub == 0:
                        stage_t[(tt, k)] = stg.tile(
                            [128, 512], bf16, name="sg", tag="sg"
                        )
                    nc.vector.tensor_copy(
                        stage_t[(tt, k)][:, sub * 256 : (sub + 1) * 256], ps[:]
                    )
                    if sub == 1:
                        nc.sync.dma_start(
                            out[tt * 128 : (tt + 1) * 128, k * 512 : (k + 1) * 512],
                            stage_t[(tt, k)][:],
                        )

                # ------- stream schedule: slot -> list of thunks -------
                from collections import defaultdict

                stream = defaultdict(list)

                def sched(slot, fn):
                    stream[slot].append(fn)

                # v tiles: tile t needed by PV(t) at slot t+2
                for t in range(TOKT):
                    sched(t, lambda t=t: v_chain(t, acc, "acc"))
                # kT cols 1..3: col c needed by S(4c)
                for c2 in (1, 2, 3):
                    for mt in range(2):
                        sched(4 * (c2 - 1) + 2 * mt, lambda mt=mt, c2=c2:
                              kq_chain(kT_sb, wk_sb, 1, mt, c2, acc, "acc"))
                # qT for qt 1..3: needed by S(16qt)
                for q2 in (1, 2, 3):
                    for mt in range(2):
                        sched(16 * q2 - 6 + 2 * mt, lambda mt=mt, q2=q2:
                              kq_chain(qT_sb, wq_sb, 0, mt, q2, acc, "acc"))
                # out-proj quarters for qt spread across qt+1's slots so
                # every slot's PE load sits just under the ACT (exp) pace
                OPQ_SLOTS = (2, 3, 3, 4, 5, 6, 6, 7, 8, 9, 9, 10, 11, 14, 14, 15)
                for q2 in (0, 1, 2):
                    for i, (qb, nq) in enumerate(
                        [(qb, nq) for qb in range(4) for nq in range(4)]
                    ):
                        sched(16 * (q2 + 1) + OPQ_SLOTS[i],
                              lambda q2=q2, qb=qb, nq=nq: emit_opq(q2, qb, nq))

                # ------- the slot loop -------
                for s in range(NSLOT):
                    emit_S(s)
                    if s >= 2:
                        emit_PV(s - 2)
                    if s % 16 == 1 and s >= 17:
                        emit_norm(s // 16 - 1)
                    for fn in stream.pop(s, ()):
                        fn()

                # ------- tail -------
                emit_PV(NSLOT - 2)
                ptl = pt_t[(NSLOT - 1, 1)]
                emit_PV(NSLOT - 1)
                # keep the PE p-state hot through the norm/transpose latency;
                # lhsT=pt of the last slot anchors these after the last exp
                # (the scheduler sinks dependency-free instructions)
                wu2 = acc.tile([128, 512], f32, name="op", tag="acc")
                for _ in range(6):
                    nc.tensor.matmul(
                        wu2[:], lhsT=ptl[:, 0, 0:128], rhs=wu_src[:],
                        start=True, stop=True,
                    )
                emit_norm(QT - 1)
                for i, (qb, ntb) in enumerate(
                    [(qb, ntb) for qb in range(4) for ntb in range(2)]
                ):
                    emit_op(QT - 1, qb, ntb, big=(i % 3 != 0),
                            store_eng=(nc.scalar if qb % 2 else nc.sync), tail=True)
    nc.finalize()
    return nc


def make_in_maps(x, W_qkv, b_qkv, W_o):
    """Shard full inputs into per-core input maps (core c: batch c//4, group c%4)."""
    import ml_dtypes

    bf = ml_dtypes.bfloat16
    x = np.asarray(x, dtype=np.float32)
    W_qkv = np.asarray(W_qkv, dtype=np.float32)
    b_qkv = np.asarray(b_qkv, dtype=np.float32)
    W_o = np.asarray(W_o, dtype=np.float32)
    in_maps = []
    for c in range(NCORES):
        b, g = divmod(c, GROUPS)
        cols = slice(CH * g, CH * (g + 1))
        bq = b_qkv[0 * D : 1 * D][cols]
        bk = b_qkv[1 * D : 2 * D][cols]
        bv = b_qkv[2 * D : 3 * D][cols]
        bqk_t = np.empty((128, 2, 2), dtype=np.float32)
        for mt in range(2):
            bqk_t[:, mt, 0] = bq[mt * 128 : (mt + 1) * 128]
            bqk_t[:, mt, 1] = bk[mt * 128 : (mt + 1) * 128]
        m = {
            "xT": np.ascontiguousarray(x[b].T).astype(bf),
            "wq": np.ascontiguousarray(W_qkv[:, 0 * D : 1 * D][:, cols]).astype(bf),
            "wk": np.ascontiguousarray(W_qkv[:, 1 * D : 2 * D][:, cols]).astype(bf),
            "wv": np.ascontiguousarray(W_qkv[:, 2 * D : 3 * D][:, cols]).astype(bf),
            "wvb": np.ascontiguousarray(bv[None, :]).astype(bf),
            "wo": np.ascontiguousarray(W_o[cols, :]).astype(bf),
            "bqk": bqk_t,
            "ones_row": np.ones((1, 128), dtype=bf),
            "ones_col": np.ones((128, 1), dtype=bf),
            "ident": np.eye(128, dtype=np.float32).astype(bf),
        }
        in_maps.append(m)
    return in_maps


_NC = None


def get_nc():
    global _NC
    if _NC is None:
        _NC = build_nc()
    return _NC


def kernel(x, W_qkv, b_qkv, W_o, b_o):
    from concourse import bass_utils

    b_o = np.asarray(b_o, dtype=np.float32)
    in_maps = make_in_maps(x, W_qkv, b_qkv, W_o)
    res = bass_utils.run_bass_kernel_spmd(get_nc(), in_maps, core_ids=list(range(NCORES)))
    out = np.empty((B, N, D), dtype=np.float32)
    for b in range(B):
        acc = np.asarray(res.results[4 * b]["out"], dtype=np.float32)
        for g in range(1, GROUPS):
            acc = acc + np.asarray(res.results[4 * b + g]["out"], dtype=np.float32)
        out[b] = acc + b_o
    return out
